# revision 52
# baseline (speedup 1.0000x reference)
"""Trainium2 Bass kernel for nn_Correlation (81-displacement cost volume).

corr(b, d, y, x) = sum_c f1[b,c,y,x] * f2[b,c,y+dy,x+dx],  d = (dy+4)*9 + (dx+4)

Sharding: data-parallel over batch B=8, one batch per NeuronCore.

Per-core algorithm (all matmuls bf16, PSUM fp32):
  Tile the (y, x) output plane into blocks of G=16 y-rows x A=8 x-cols.
  For block (g, cc) the PE computes, per channel-half ch (K=128 each):
      psum[m=(s,xi), n=(row,xw)] += f1[c, y=16g+s, x=8cc+xi] *
                                    f2p[c, yp=16g+row, xp=8cc+xw]
  with s in [0,16), xi in [0,8)  (M = 128 weights = one f1 block)
  and row in [0,24), xw in [0,16) (N = 384 = the 24x16 padded f2 window).
  Entry (s,xi,row,xw) equals corr(y=16g+s, x=8cc+xi, dy=row-s, dx=xw-xi).

Pipeline engineering (v6):
  - The reference zero-pads fmap2, so out-of-range displacements are
    EXACTLY zero in the output.  The kernel therefore loads f2 with no
    padding at all (flat [y,x] per channel-half, 4 garbage rows of SBUF
    above/below, x windows wrap across row boundaries into garbage) and
    the host gather masks out-of-range entries to zero.  Saves the pad
    memsets and 6% of f2 traffic.
  - Inputs stream on both HWDGE rings (SP + ACT), interleaved in
    deadline order so each y-block's f1 and f2 land just before the PE
    needs them; g0's first f1 half-tile is filled by two quarter DMAs so
    the first LDWEIGHTS gates on 0.25 MB.
  - DVE/ACT alternate evacuating psum into a per-y-block bf16 stage tile
    in hybrid layout [128, row(24), cc(16), xw(16)].  Group j (partitions
    32j..32j+32) only needs rows [4j, 4j+12) = one contiguous 3072-elem
    slab.  4 slab DMAs per y-block, all on the SP ring except the last
    y-block's odd groups which go on the ACT ring (idle by then).
  - No gpsimd DMAs (SWDGE drain is slow in the epilogue).
  The fine per-lane gather (s%4+dy, cc, xi+dx) happens on host.
"""

import sys

sys.path.insert(0, "/opt/trn_rl_repo")

from contextlib import ExitStack

import ml_dtypes
import numpy as np

import jax

jax.config.update("jax_compilation_cache_dir", "/root/jaxcache")
jax.config.update("jax_persistent_cache_min_entry_size_bytes", 0)
jax.config.update("jax_persistent_cache_min_compile_time_secs", 0)

import concourse.bass as bass
import concourse.tile as tile
from concourse import bacc, mybir
from concourse.ap import AP
from concourse.bass_utils import run_bass_kernel_spmd

F32 = mybir.dt.float32
BF16 = mybir.dt.bfloat16
BF16_NP = ml_dtypes.bfloat16

B = 8
C = 256
H = 64
W = 128
PAD = 4
G = 16       # y rows per block
A = 8        # x cols per block
NG = H // G  # 4 y-blocks
NC_ = W // A  # 16 x-blocks
HC = NC_ // 2  # 8 x-blocks per f1 half-tile
ROWS = G + 2 * PAD   # 24 window rows per block
WIN = A + 2 * PAD    # 16 window cols per block
NMM = ROWS * WIN     # 384 psum columns per block
HP = H + 2 * PAD     # 72 tile rows per channel-half (4 garbage top/bottom)
NB = 81
SLAB = (2 * PAD + 4) * WIN  # 192 e-rows per 32-partition group (12 rows)

F1H = 2 * HC * G * A         # 2048 elems per partition per f1 half-tile
CHB = PAD + HP * W + 124     # per-ch block stride in the f2 tile (slack both ends)
F2SZ = 2 * CHB + 112         # f2 tile free size (last window slice overruns)


def build_program():
    nc = bacc.Bacc("TRN2", target_bir_lowering=False, debug=False)

    # f1 half-tiles: [g, half, cpart, cc_local*ch*s*xi] (cc-major!)
    f1r_d = nc.dram_tensor("f1r", [NG, 2, 128, F1H], BF16, kind="ExternalInput").ap()
    f2_d = nc.dram_tensor("f2", [128, 2, H, W], BF16, kind="ExternalInput").ap()
    s1_d = nc.dram_tensor(
        "s1", [128, NG, SLAB * NC_], BF16, kind="ExternalOutput"
    ).ap()

    # real-row chunks, in the order the y-blocks consume them
    CHUNKS = [(0, 20), (20, 36), (36, 52), (52, 64)]

    with tile.TileContext(nc) as tc, ExitStack() as ctx:
        f2_pool = ctx.enter_context(tc.tile_pool(name="f2", bufs=1))
        f1_pool = ctx.enter_context(tc.tile_pool(name="f1", bufs=2 * NG))
        stage_pool = ctx.enter_context(tc.tile_pool(name="stage", bufs=NG))
        psum_pool = ctx.enter_context(tc.tile_pool(name="ps", bufs=8, space="PSUM"))

        f2_t = f2_pool.tile([128, F2SZ], BF16)
        f2_f = f2_t[:]

        # PE p-state warmup: ~8 dummy matmuls on scratch data keep the PE
        # continuously busy from ~9 us so the real stream starts at the
        # full 2.4 GHz clock instead of ramping through it
        wsrc_pool = ctx.enter_context(tc.tile_pool(name="wsrc", bufs=1))
        wsrc = wsrc_pool.tile([128, 640], BF16)
        nc.gpsimd.memset(wsrc[:], 0.0)
        wps = psum_pool.tile([128, 512], F32, tag="ps", name="warm_ps")
        for _ in range(8):
            nc.tensor.matmul(
                wps[:], wsrc[:, :128], wsrc[:, 128:640], start=True, stop=True
            )

        def chunk_dma(eng, g, ch, rows=None):
            lo, hi = CHUNKS[g]
            if rows is not None:
                lo, hi = rows
            dst = f2_f[:, CHB * ch + PAD + (PAD + lo) * W : CHB * ch + PAD + (PAD + hi) * W]
            eng.dma_start(dst, f2_d[:, ch, lo:hi, :])

        def chunk_dma2(eng, g):
            # both channel-halves in one DMA (3-dim pattern, ch stride CHB)
            lo, hi = CHUNKS[g]
            n = (hi - lo) * W
            dst = AP(
                f2_f.tensor,
                f2_f.offset + PAD + (PAD + lo) * W,
                [[F2SZ, 128], [CHB, 2], [1, n]],
            )
            eng.dma_start(dst, f2_d[:, :, lo:hi, :])

        f1_tiles = [[None, None] for _ in range(NG)]

        def f1_dma(eng, g, h, quarter=None):
            if f1_tiles[g][h] is None:
                f1_tiles[g][h] = f1_pool.tile(
                    [128, F1H], BF16, tag="f1g", name=f"f1_{g}_{h}"
                )
            t = f1_tiles[g][h]
            if quarter is None:
                eng.dma_start(t[:], f1r_d[g, h])
            elif quarter == 0:
                eng.dma_start(t[:, : F1H // 2], f1r_d[g, h][:, : F1H // 2])
            else:
                eng.dma_start(t[:, F1H // 2 :], f1r_d[g, h][:, F1H // 2 :])

        # deadline-ordered input streams; each ring is FIFO at ~half the
        # fabric, so per-ring cumulative bytes track each block's deadline.
        # Few DMAs total keeps the 8-sem round-robin pool shallow (fewer
        # recycle chains).
        sy, sc = nc.sync, nc.scalar
        chunk_dma(sy, 0, 0)                  # g0 ch0 window
        chunk_dma(sc, 0, 1)                  # g0 ch1 window
        f1_dma(sy, 0, 0, quarter=0)          # cc0-3 weights: MM0 gate
        f1_dma(sc, 0, 0, quarter=1)          # cc4-7
        f1_dma(sy, 0, 1, quarter=0)          # cc8-11
        f1_dma(sc, 0, 1, quarter=1)          # cc12-15
        chunk_dma(sy, 1, 0)
        chunk_dma(sc, 1, 1)
        f1_dma(sy, 1, 0)
        f1_dma(sc, 1, 1)
        chunk_dma(sy, 2, 0)
        chunk_dma(sc, 2, 1)
        f1_dma(sy, 2, 0)
        f1_dma(sc, 2, 1)
        chunk_dma(sy, 3, 0)
        chunk_dma(sc, 3, 1)
        f1_dma(sy, 3, 0)
        f1_dma(sc, 3, 1)

        def window(g, ch, cc, rlo, rhi):
            # rows 16g+rlo..16g+rhi, x cols 8cc-4..8cc+12 (wrapping into
            # garbage).  The first/last y-blocks skip the window rows that
            # only produce out-of-range (host-masked) displacements.
            start = CHB * ch + PAD + (16 * g + rlo) * W + 8 * cc - PAD
            nr = rhi - rlo
            sl = f2_f[:, start : start + nr * W]
            return sl.rearrange("p (r x) -> p r x", r=nr)[:, :, :WIN]

        for g in range(NG):
            # f1 half layout [cc_local, ch, s, xi]: the (s, xi) weight
            # block for one (cc, ch) is contiguous, as LDWEIGHTS requires
            f1_vs = [
                f1_tiles[g][h][:].rearrange(
                    "p (t c s x) -> p t c (s x)", t=HC, c=2, s=G
                )
                for h in range(2)
            ]
            stage_t = stage_pool.tile(
                [128, NMM * NC_], BF16, tag="stg", name=f"stg_{g}"
            )
            # hybrid layout [row(24), t(16), xw(16)]: evac writes 16-elem
            # contiguous runs; each group-j slab is one contiguous 3072-run
            stage_e = stage_t[:].rearrange("p (r t w) -> p r t w", r=ROWS, t=NC_)

            # skip window rows whose displacements are all out of range
            # (host-masked): 4 at the top of g0, 4 at the bottom of g3
            rlo = PAD if g == 0 else 0
            rhi = ROWS - PAD if g == NG - 1 else ROWS
            ncols = (rhi - rlo) * WIN

            def mm(cc, ch, ps):
                nc.tensor.matmul(
                    ps[:],
                    f1_vs[cc // HC][:, cc % HC, ch, :],
                    window(g, ch, cc, rlo, rhi),
                    start=(ch == 0),
                    stop=(ch == 1),
                )

            def evac(cc, ps):
                dst = stage_e[:, rlo:rhi, cc, :]
                if cc % 2 == 0:
                    nc.vector.tensor_copy(dst, ps[:])
                else:
                    nc.scalar.copy(dst, ps[:])

            psums = {}
            if g == 0:
                # ch0 of the first 8 x-blocks first: they gate only on the
                # f1g0a quarters + the ch0 chunk halves, bridging the PE
                # until the ch1 chunk lands (~2.6 us later)
                for cc in range(8):
                    psums[cc] = psum_pool.tile([128, ncols], F32, tag="ps", name=f"ps0_{cc}")
                    mm(cc, 0, psums[cc])
                for cc in range(8):
                    mm(cc, 1, psums[cc])
                    evac(cc, psums[cc])
                rest = range(8, NC_)
            else:
                rest = range(NC_)
            for cc in rest:
                ps = psum_pool.tile([128, ncols], F32, tag="ps", name=f"ps_{g}_{cc}")
                mm(cc, 0, ps)
                mm(cc, 1, ps)
                evac(cc, ps)

            # contiguous per-32-partition-group slabs (group j needs rows
            # [4j, 4j+12), one 3072-elem run per partition).  All on the SP
            # ring except the last y-block, which splits across SP/ACT; the
            # scalar pair is emitted first so any semaphore the allocator
            # recycles between adjacent slabs lands within one engine
            # (where triggers serialize anyway) instead of across engines.
            def slab(eng, j):
                eng.dma_start(
                    s1_d[32 * j : 32 * j + 32, g, :],
                    stage_t[
                        32 * j : 32 * j + 32,
                        64 * j * NC_ : (64 * j + SLAB) * NC_,
                    ],
                )

            if g == NG - 1:
                # split the final 0.79 MB across both rings
                slab(nc.scalar, 1)
                slab(nc.sync, 0)
                slab(nc.scalar, 3)
                slab(nc.sync, 2)
            else:
                for j in range(4):
                    slab(nc.sync, j)

    nc.compile()
    return nc


def prep_inputs(fmap1: np.ndarray, fmap2: np.ndarray):
    f1 = np.asarray(fmap1, dtype=np.float32).reshape(B, 2, 128, NG, G, NC_, A)
    # f1r[b, g, half, cpart, cc_local, ch, s, xi] (cc-major within a half)
    f1r = (
        np.ascontiguousarray(f1.transpose(0, 3, 2, 1, 5, 4, 6))  # b g cp ch cc s xi
        .astype(BF16_NP)
        .reshape(B, NG, 128, 2, 2, HC, G, A)  # b g cp ch half ccl s xi
        .transpose(0, 1, 4, 2, 5, 3, 6, 7)    # b g half cp ccl ch s xi
        .reshape(B, NG, 2, 128, F1H)
    )
    f1r = np.ascontiguousarray(f1r)
    f2 = np.asarray(fmap2, dtype=np.float32).reshape(B, 2, 128, H, W)
    # f2r[b, cpart, ch, y, x] - no padding
    f2r = np.ascontiguousarray(f2.transpose(0, 2, 1, 3, 4)).astype(BF16_NP)
    return f1r, f2r


def _host_gather_idx():
    y = np.arange(H)
    x = np.arange(W)
    g = y // G
    s = y % G
    cc = x // A
    xi = x % A
    p = (8 * s)[:, None] + xi[None, :]          # [H, W]
    dyg = np.arange(NB) // 9
    dxg = np.arange(NB) % 9
    # slab-local offset: (s%4+dy)*(NC_*WIN) + cc*WIN + xi + dx
    e_rel = (
        ((s % 4)[None, :, None] + dyg[:, None, None]) * (NC_ * WIN)
        + cc[None, None, :] * WIN
        + xi[None, None, :]
        + dxg[:, None, None]
    )                                            # [81, H, W]
    flat = (p[None] * NG + g[None, :, None]) * (SLAB * NC_) + e_rel
    # out-of-range displacements correlate against the reference's zero
    # padding -> exactly zero; the kernel leaves garbage there, mask it.
    yy = y[None, :, None] + dyg[:, None, None] - PAD
    xx = x[None, None, :] + dxg[:, None, None] - PAD
    valid = (yy >= 0) & (yy < H) & (xx >= 0) & (xx < W)
    return flat.reshape(-1), valid.reshape(-1)


_FLAT_IDX, _VALID = _host_gather_idx()


def finish_host(s1_all: np.ndarray) -> np.ndarray:
    s1 = np.asarray(s1_all, dtype=np.float32).reshape(B, -1)
    out = np.where(_VALID[None], s1[:, _FLAT_IDX], np.float32(0.0))
    return out.reshape(B, NB, H, W)


_CACHE = {}


def _get_program():
    if "p" not in _CACHE:
        _CACHE["p"] = build_program()
    return _CACHE["p"]


def run_on_cores(fmap1, fmap2, trace=False):
    nc = _get_program()
    f1r, f2r = prep_inputs(fmap1, fmap2)
    in_maps = [{"f1r": f1r[b], "f2": f2r[b]} for b in range(B)]
    res = run_bass_kernel_spmd(nc, in_maps, core_ids=list(range(B)), trace=trace)
    s1_all = np.stack([res.results[b]["s1"] for b in range(B)], axis=0)
    out = finish_host(s1_all)
    return out, res


def kernel(fmap1: np.ndarray, fmap2: np.ndarray) -> np.ndarray:
    fmap1 = np.asarray(fmap1, dtype=np.float32)
    fmap2 = np.asarray(fmap2, dtype=np.float32)
    out, _ = run_on_cores(fmap1, fmap2, trace=False)
    return out


# revision 54
# speedup vs baseline: 1.0121x; 1.0121x over previous
"""Trainium2 Bass kernel for nn_Correlation (81-displacement cost volume).

corr(b, d, y, x) = sum_c f1[b,c,y,x] * f2[b,c,y+dy,x+dx],  d = (dy+4)*9 + (dx+4)

Sharding: data-parallel over batch B=8, one batch per NeuronCore.

Per-core algorithm (all matmuls bf16, PSUM fp32):
  Tile the (y, x) output plane into blocks of G=16 y-rows x A=8 x-cols.
  For block (g, cc) the PE computes, per channel-half ch (K=128 each):
      psum[m=(s,xi), n=(row,xw)] += f1[c, y=16g+s, x=8cc+xi] *
                                    f2p[c, yp=16g+row, xp=8cc+xw]
  with s in [0,16), xi in [0,8)  (M = 128 weights = one f1 block)
  and row in [0,24), xw in [0,16) (N = 384 = the 24x16 padded f2 window).
  Entry (s,xi,row,xw) equals corr(y=16g+s, x=8cc+xi, dy=row-s, dx=xw-xi).

Pipeline engineering (v6):
  - The reference zero-pads fmap2, so out-of-range displacements are
    EXACTLY zero in the output.  The kernel therefore loads f2 with no
    padding at all (flat [y,x] per channel-half, 4 garbage rows of SBUF
    above/below, x windows wrap across row boundaries into garbage) and
    the host gather masks out-of-range entries to zero.  Saves the pad
    memsets and 6% of f2 traffic.
  - Inputs stream on both HWDGE rings (SP + ACT), interleaved in
    deadline order so each y-block's f1 and f2 land just before the PE
    needs them; g0's first f1 half-tile is filled by two quarter DMAs so
    the first LDWEIGHTS gates on 0.25 MB.
  - DVE/ACT alternate evacuating psum into a per-y-block bf16 stage tile
    in hybrid layout [128, row(24), cc(16), xw(16)].  Group j (partitions
    32j..32j+32) only needs rows [4j, 4j+12) = one contiguous 3072-elem
    slab.  4 slab DMAs per y-block, all on the SP ring except the last
    y-block's odd groups which go on the ACT ring (idle by then).
  - No gpsimd DMAs (SWDGE drain is slow in the epilogue).
  The fine per-lane gather (s%4+dy, cc, xi+dx) happens on host.
"""

import sys

sys.path.insert(0, "/opt/trn_rl_repo")

from contextlib import ExitStack

import ml_dtypes
import numpy as np

import jax

jax.config.update("jax_compilation_cache_dir", "/root/jaxcache")
jax.config.update("jax_persistent_cache_min_entry_size_bytes", 0)
jax.config.update("jax_persistent_cache_min_compile_time_secs", 0)

import concourse.bass as bass
import concourse.tile as tile
from concourse import bacc, mybir
from concourse.ap import AP
from concourse.bass_utils import run_bass_kernel_spmd

F32 = mybir.dt.float32
BF16 = mybir.dt.bfloat16
BF16_NP = ml_dtypes.bfloat16

B = 8
C = 256
H = 64
W = 128
PAD = 4
G = 16       # y rows per block
A = 8        # x cols per block
NG = H // G  # 4 y-blocks
NC_ = W // A  # 16 x-blocks
HC = NC_ // 2  # 8 x-blocks per f1 half-tile
ROWS = G + 2 * PAD   # 24 window rows per block
WIN = A + 2 * PAD    # 16 window cols per block
NMM = ROWS * WIN     # 384 psum columns per block
HP = H + 2 * PAD     # 72 tile rows per channel-half (4 garbage top/bottom)
NB = 81
SLAB = (2 * PAD + 4) * WIN  # 192 e-rows per 32-partition group (12 rows)

F1H = 2 * HC * G * A         # 2048 elems per partition per f1 half-tile
CHB = PAD + HP * W + 124     # per-ch block stride in the f2 tile (slack both ends)
F2SZ = 2 * CHB + 112         # f2 tile free size (last window slice overruns)


def build_program():
    nc = bacc.Bacc("TRN2", target_bir_lowering=False, debug=False)

    # f1 half-tiles: [g, half, cpart, cc_local*ch*s*xi] (cc-major!)
    f1r_d = nc.dram_tensor("f1r", [NG, 2, 128, F1H], BF16, kind="ExternalInput").ap()
    f2_d = nc.dram_tensor("f2", [128, 2, H, W], BF16, kind="ExternalInput").ap()
    s1_d = nc.dram_tensor(
        "s1", [128, NG, SLAB * NC_], BF16, kind="ExternalOutput"
    ).ap()

    # real-row chunks, in the order the y-blocks consume them
    CHUNKS = [(0, 20), (20, 36), (36, 52), (52, 64)]

    with tile.TileContext(nc) as tc, ExitStack() as ctx:
        f2_pool = ctx.enter_context(tc.tile_pool(name="f2", bufs=1))
        f1_pool = ctx.enter_context(tc.tile_pool(name="f1", bufs=2 * NG))
        stage_pool = ctx.enter_context(tc.tile_pool(name="stage", bufs=NG))
        psum_pool = ctx.enter_context(tc.tile_pool(name="ps", bufs=4, space="PSUM"))

        f2_t = f2_pool.tile([128, F2SZ], BF16)
        f2_f = f2_t[:]

        # PE p-state warmup: ~8 dummy matmuls on scratch data keep the PE
        # continuously busy from ~9 us so the real stream starts at the
        # full 2.4 GHz clock instead of ramping through it
        wsrc_pool = ctx.enter_context(tc.tile_pool(name="wsrc", bufs=1))
        wsrc = wsrc_pool.tile([128, 640], BF16)
        nc.gpsimd.memset(wsrc[:], 0.0)
        wps = psum_pool.tile([128, 512], F32, tag="ps", name="warm_ps")
        for _ in range(8):
            nc.tensor.matmul(
                wps[:], wsrc[:, :128], wsrc[:, 128:640], start=True, stop=True
            )

        def chunk_dma(eng, g, ch, rows=None):
            lo, hi = CHUNKS[g]
            if rows is not None:
                lo, hi = rows
            dst = f2_f[:, CHB * ch + PAD + (PAD + lo) * W : CHB * ch + PAD + (PAD + hi) * W]
            eng.dma_start(dst, f2_d[:, ch, lo:hi, :])

        def chunk_dma2(eng, g):
            # both channel-halves in one DMA (3-dim pattern, ch stride CHB)
            lo, hi = CHUNKS[g]
            n = (hi - lo) * W
            dst = AP(
                f2_f.tensor,
                f2_f.offset + PAD + (PAD + lo) * W,
                [[F2SZ, 128], [CHB, 2], [1, n]],
            )
            eng.dma_start(dst, f2_d[:, :, lo:hi, :])

        f1_tiles = [[None, None] for _ in range(NG)]

        def f1_dma(eng, g, h, quarter=None):
            if f1_tiles[g][h] is None:
                f1_tiles[g][h] = f1_pool.tile(
                    [128, F1H], BF16, tag="f1g", name=f"f1_{g}_{h}"
                )
            t = f1_tiles[g][h]
            if quarter is None:
                eng.dma_start(t[:], f1r_d[g, h])
            elif quarter == 0:
                eng.dma_start(t[:, : F1H // 2], f1r_d[g, h][:, : F1H // 2])
            else:
                eng.dma_start(t[:, F1H // 2 :], f1r_d[g, h][:, F1H // 2 :])

        # deadline-ordered input streams; each ring is FIFO at ~half the
        # fabric, so per-ring cumulative bytes track each block's deadline.
        # Few DMAs total keeps the 8-sem round-robin pool shallow (fewer
        # recycle chains).
        sy, sc = nc.sync, nc.scalar
        chunk_dma(sy, 0, 0)                  # g0 ch0 window
        chunk_dma(sc, 0, 1)                  # g0 ch1 window
        f1_dma(sy, 0, 0, quarter=0)          # cc0-3 weights: MM0 gate
        f1_dma(sc, 0, 0, quarter=1)          # cc4-7
        f1_dma(sy, 0, 1, quarter=0)          # cc8-11
        f1_dma(sc, 0, 1, quarter=1)          # cc12-15
        chunk_dma(sy, 1, 0)
        chunk_dma(sc, 1, 1)
        f1_dma(sy, 1, 0)
        f1_dma(sc, 1, 1)
        chunk_dma(sy, 2, 0)
        chunk_dma(sc, 2, 1)
        f1_dma(sy, 2, 0)
        f1_dma(sc, 2, 1)
        chunk_dma(sy, 3, 0)
        chunk_dma(sc, 3, 1)
        f1_dma(sy, 3, 0)
        f1_dma(sc, 3, 1)

        def window(g, ch, cc, rlo, rhi):
            # rows 16g+rlo..16g+rhi, x cols 8cc-4..8cc+12 (wrapping into
            # garbage).  The first/last y-blocks skip the window rows that
            # only produce out-of-range (host-masked) displacements.
            start = CHB * ch + PAD + (16 * g + rlo) * W + 8 * cc - PAD
            nr = rhi - rlo
            sl = f2_f[:, start : start + nr * W]
            return sl.rearrange("p (r x) -> p r x", r=nr)[:, :, :WIN]

        for g in range(NG):
            # f1 half layout [cc_local, ch, s, xi]: the (s, xi) weight
            # block for one (cc, ch) is contiguous, as LDWEIGHTS requires
            f1_vs = [
                f1_tiles[g][h][:].rearrange(
                    "p (t c s x) -> p t c (s x)", t=HC, c=2, s=G
                )
                for h in range(2)
            ]
            stage_t = stage_pool.tile(
                [128, NMM * NC_], BF16, tag="stg", name=f"stg_{g}"
            )
            # hybrid layout [row(24), t(16), xw(16)]: evac writes 16-elem
            # contiguous runs; each group-j slab is one contiguous 3072-run
            stage_e = stage_t[:].rearrange("p (r t w) -> p r t w", r=ROWS, t=NC_)

            # skip window rows whose displacements are all out of range
            # (host-masked): 4 at the top of g0, 4 at the bottom of g3
            rlo = PAD if g == 0 else 0
            rhi = ROWS - PAD if g == NG - 1 else ROWS
            nrows = rhi - rlo
            ncols = nrows * WIN

            def mm(cc, ch, dst):
                nc.tensor.matmul(
                    dst,
                    f1_vs[cc // HC][:, cc % HC, ch, :],
                    window(g, ch, cc, rlo, rhi),
                    start=(ch == 0),
                    stop=(ch == 1),
                )

            # two x-blocks share one 2-bank psum tile; a single 4-dim copy
            # evacuates both (halves the per-op fixed overhead, giving the
            # evac lane margin over the PE's production rate)
            def pair_tile(name):
                return psum_pool.tile([128, 1024], F32, tag="ps", name=name)

            def pslice(pt, half):
                return pt[:, 512 * half : 512 * half + ncols]

            def evac_pair(cc, pt):
                base = pt[:]
                src = AP(
                    base.tensor,
                    base.offset,
                    [[1024, 128], [WIN, nrows], [512, 2], [1, WIN]],
                )
                dst = stage_e[:, rlo:rhi, cc : cc + 2, :]
                if (cc // 2) % 2 == 0:
                    nc.vector.tensor_copy(dst, src)
                else:
                    nc.scalar.copy(dst, src)

            if g == 0:
                # ch0 of the first 8 x-blocks first: they gate only on the
                # f1g0a quarters + the ch0 chunk halves, bridging the PE
                # until the ch1 chunk lands (~2.6 us later)
                pts = [pair_tile(f"pp0_{pcc}") for pcc in range(0, 8, 2)]
                for cc in range(8):
                    mm(cc, 0, pslice(pts[cc // 2], cc % 2))
                for cc in range(8):
                    mm(cc, 1, pslice(pts[cc // 2], cc % 2))
                    if cc % 2 == 1:
                        evac_pair(cc - 1, pts[cc // 2])
                rest = range(8, NC_, 2)
            else:
                rest = range(0, NC_, 2)
            for pcc in rest:
                pt = pair_tile(f"pp_{g}_{pcc}")
                mm(pcc, 0, pslice(pt, 0))
                mm(pcc, 1, pslice(pt, 0))
                mm(pcc + 1, 0, pslice(pt, 1))
                mm(pcc + 1, 1, pslice(pt, 1))
                evac_pair(pcc, pt)

            # contiguous per-32-partition-group slabs (group j needs rows
            # [4j, 4j+12), one 3072-elem run per partition).  All on the SP
            # ring except the last y-block, which splits across SP/ACT; the
            # scalar pair is emitted first so any semaphore the allocator
            # recycles between adjacent slabs lands within one engine
            # (where triggers serialize anyway) instead of across engines.
            def slab(eng, j):
                eng.dma_start(
                    s1_d[32 * j : 32 * j + 32, g, :],
                    stage_t[
                        32 * j : 32 * j + 32,
                        64 * j * NC_ : (64 * j + SLAB) * NC_,
                    ],
                )

            if g == NG - 1:
                # split the final 0.79 MB across both rings
                slab(nc.scalar, 1)
                slab(nc.sync, 0)
                slab(nc.scalar, 3)
                slab(nc.sync, 2)
            else:
                for j in range(4):
                    slab(nc.sync, j)

    nc.compile()
    return nc


def prep_inputs(fmap1: np.ndarray, fmap2: np.ndarray):
    f1 = np.asarray(fmap1, dtype=np.float32).reshape(B, 2, 128, NG, G, NC_, A)
    # f1r[b, g, half, cpart, cc_local, ch, s, xi] (cc-major within a half)
    f1r = (
        np.ascontiguousarray(f1.transpose(0, 3, 2, 1, 5, 4, 6))  # b g cp ch cc s xi
        .astype(BF16_NP)
        .reshape(B, NG, 128, 2, 2, HC, G, A)  # b g cp ch half ccl s xi
        .transpose(0, 1, 4, 2, 5, 3, 6, 7)    # b g half cp ccl ch s xi
        .reshape(B, NG, 2, 128, F1H)
    )
    f1r = np.ascontiguousarray(f1r)
    f2 = np.asarray(fmap2, dtype=np.float32).reshape(B, 2, 128, H, W)
    # f2r[b, cpart, ch, y, x] - no padding
    f2r = np.ascontiguousarray(f2.transpose(0, 2, 1, 3, 4)).astype(BF16_NP)
    return f1r, f2r


def _host_gather_idx():
    y = np.arange(H)
    x = np.arange(W)
    g = y // G
    s = y % G
    cc = x // A
    xi = x % A
    p = (8 * s)[:, None] + xi[None, :]          # [H, W]
    dyg = np.arange(NB) // 9
    dxg = np.arange(NB) % 9
    # slab-local offset: (s%4+dy)*(NC_*WIN) + cc*WIN + xi + dx
    e_rel = (
        ((s % 4)[None, :, None] + dyg[:, None, None]) * (NC_ * WIN)
        + cc[None, None, :] * WIN
        + xi[None, None, :]
        + dxg[:, None, None]
    )                                            # [81, H, W]
    flat = (p[None] * NG + g[None, :, None]) * (SLAB * NC_) + e_rel
    # out-of-range displacements correlate against the reference's zero
    # padding -> exactly zero; the kernel leaves garbage there, mask it.
    yy = y[None, :, None] + dyg[:, None, None] - PAD
    xx = x[None, None, :] + dxg[:, None, None] - PAD
    valid = (yy >= 0) & (yy < H) & (xx >= 0) & (xx < W)
    return flat.reshape(-1), valid.reshape(-1)


_FLAT_IDX, _VALID = _host_gather_idx()


def finish_host(s1_all: np.ndarray) -> np.ndarray:
    s1 = np.asarray(s1_all, dtype=np.float32).reshape(B, -1)
    out = np.where(_VALID[None], s1[:, _FLAT_IDX], np.float32(0.0))
    return out.reshape(B, NB, H, W)


_CACHE = {}


def _get_program():
    if "p" not in _CACHE:
        _CACHE["p"] = build_program()
    return _CACHE["p"]


def run_on_cores(fmap1, fmap2, trace=False):
    nc = _get_program()
    f1r, f2r = prep_inputs(fmap1, fmap2)
    in_maps = [{"f1r": f1r[b], "f2": f2r[b]} for b in range(B)]
    res = run_bass_kernel_spmd(nc, in_maps, core_ids=list(range(B)), trace=trace)
    s1_all = np.stack([res.results[b]["s1"] for b in range(B)], axis=0)
    out = finish_host(s1_all)
    return out, res


def kernel(fmap1: np.ndarray, fmap2: np.ndarray) -> np.ndarray:
    fmap1 = np.asarray(fmap1, dtype=np.float32)
    fmap2 = np.asarray(fmap2, dtype=np.float32)
    out, _ = run_on_cores(fmap1, fmap2, trace=False)
    return out


# revision 55
# speedup vs baseline: 1.0544x; 1.0417x over previous
"""Trainium2 Bass kernel for nn_Correlation (81-displacement cost volume).

corr(b, d, y, x) = sum_c f1[b,c,y,x] * f2[b,c,y+dy,x+dx],  d = (dy+4)*9 + (dx+4)

Sharding: data-parallel over batch B=8, one batch per NeuronCore.

Per-core algorithm (all matmuls bf16, PSUM fp32):
  Tile the (y, x) output plane into blocks of G=16 y-rows x A=8 x-cols.
  For block (g, cc) the PE computes, per channel-half ch (K=128 each):
      psum[m=(s,xi), n=(row,xw)] += f1[c, y=16g+s, x=8cc+xi] *
                                    f2p[c, yp=16g+row, xp=8cc+xw]
  with s in [0,16), xi in [0,8)  (M = 128 weights = one f1 block)
  and row in [0,24), xw in [0,16) (N = 384 = the 24x16 padded f2 window).
  Entry (s,xi,row,xw) equals corr(y=16g+s, x=8cc+xi, dy=row-s, dx=xw-xi).

Pipeline engineering (v6):
  - The reference zero-pads fmap2, so out-of-range displacements are
    EXACTLY zero in the output.  The kernel therefore loads f2 with no
    padding at all (flat [y,x] per channel-half, 4 garbage rows of SBUF
    above/below, x windows wrap across row boundaries into garbage) and
    the host gather masks out-of-range entries to zero.  Saves the pad
    memsets and 6% of f2 traffic.
  - Inputs stream on both HWDGE rings (SP + ACT), interleaved in
    deadline order so each y-block's f1 and f2 land just before the PE
    needs them; g0's first f1 half-tile is filled by two quarter DMAs so
    the first LDWEIGHTS gates on 0.25 MB.
  - DVE/ACT alternate evacuating psum into a per-y-block bf16 stage tile
    in hybrid layout [128, row(24), cc(16), xw(16)].  Group j (partitions
    32j..32j+32) only needs rows [4j, 4j+12) = one contiguous 3072-elem
    slab.  4 slab DMAs per y-block, all on the SP ring except the last
    y-block's odd groups which go on the ACT ring (idle by then).
  - No gpsimd DMAs (SWDGE drain is slow in the epilogue).
  The fine per-lane gather (s%4+dy, cc, xi+dx) happens on host.
"""

import sys

sys.path.insert(0, "/opt/trn_rl_repo")

from contextlib import ExitStack

import ml_dtypes
import numpy as np

import jax

jax.config.update("jax_compilation_cache_dir", "/root/jaxcache")
jax.config.update("jax_persistent_cache_min_entry_size_bytes", 0)
jax.config.update("jax_persistent_cache_min_compile_time_secs", 0)

import concourse.bass as bass
import concourse.tile as tile
from concourse import bacc, mybir
from concourse.ap import AP
from concourse.bass_utils import run_bass_kernel_spmd

F32 = mybir.dt.float32
BF16 = mybir.dt.bfloat16
BF16_NP = ml_dtypes.bfloat16

B = 8
C = 256
H = 64
W = 128
PAD = 4
G = 16       # y rows per block
A = 8        # x cols per block
NG = H // G  # 4 y-blocks
NC_ = W // A  # 16 x-blocks
HC = NC_ // 2  # 8 x-blocks per f1 half-tile
ROWS = G + 2 * PAD   # 24 window rows per block
WIN = A + 2 * PAD    # 16 window cols per block
NMM = ROWS * WIN     # 384 psum columns per block
HP = H + 2 * PAD     # 72 tile rows per channel-half (4 garbage top/bottom)
NB = 81
SLAB = (2 * PAD + 4) * WIN  # 192 e-rows per 32-partition group (12 rows)

F1H = 2 * HC * G * A         # 2048 elems per partition per f1 half-tile
CHB = PAD + HP * W + 124     # per-ch block stride in the f2 tile (slack both ends)
F2SZ = 2 * CHB + 112         # f2 tile free size (last window slice overruns)


def build_program():
    nc = bacc.Bacc("TRN2", target_bir_lowering=False, debug=False)

    # f1 half-tiles: [g, half, cpart, cc_local*ch*s*xi] (cc-major!)
    f1r_d = nc.dram_tensor("f1r", [NG, 2, 128, F1H], BF16, kind="ExternalInput").ap()
    f2_d = nc.dram_tensor("f2", [128, 2, H, W], BF16, kind="ExternalInput").ap()
    s1_d = nc.dram_tensor(
        "s1", [128, NG, SLAB * NC_], BF16, kind="ExternalOutput"
    ).ap()

    # real-row chunks, in the order the y-blocks consume them
    CHUNKS = [(0, 20), (20, 36), (36, 52), (52, 64)]

    with tile.TileContext(nc) as tc, ExitStack() as ctx:
        f2_pool = ctx.enter_context(tc.tile_pool(name="f2", bufs=1))
        f1_pool = ctx.enter_context(tc.tile_pool(name="f1", bufs=2 * NG))
        stage_pool = ctx.enter_context(tc.tile_pool(name="stage", bufs=NG))
        psum_pool = ctx.enter_context(tc.tile_pool(name="ps", bufs=4, space="PSUM"))

        f2_t = f2_pool.tile([128, F2SZ], BF16)
        f2_f = f2_t[:]

        # PE p-state warmup: ~8 dummy matmuls on scratch data keep the PE
        # continuously busy from ~9 us so the real stream starts at the
        # full 2.4 GHz clock instead of ramping through it
        wsrc_pool = ctx.enter_context(tc.tile_pool(name="wsrc", bufs=1))
        wsrc = wsrc_pool.tile([128, 640], BF16)
        nc.gpsimd.memset(wsrc[:], 0.0)
        wps = psum_pool.tile([128, 512], F32, tag="ps", name="warm_ps")
        for _ in range(12):
            nc.tensor.matmul(
                wps[:], wsrc[:, :128], wsrc[:, 128:640], start=True, stop=True
            )

        def chunk_dma(eng, g, ch, rows=None):
            lo, hi = CHUNKS[g]
            if rows is not None:
                lo, hi = rows
            dst = f2_f[:, CHB * ch + PAD + (PAD + lo) * W : CHB * ch + PAD + (PAD + hi) * W]
            eng.dma_start(dst, f2_d[:, ch, lo:hi, :])

        def chunk_dma2(eng, g):
            # both channel-halves in one DMA (3-dim pattern, ch stride CHB)
            lo, hi = CHUNKS[g]
            n = (hi - lo) * W
            dst = AP(
                f2_f.tensor,
                f2_f.offset + PAD + (PAD + lo) * W,
                [[F2SZ, 128], [CHB, 2], [1, n]],
            )
            eng.dma_start(dst, f2_d[:, :, lo:hi, :])

        f1_tiles = [[None, None] for _ in range(NG)]

        def f1_dma(eng, g, h, quarter=None):
            if f1_tiles[g][h] is None:
                f1_tiles[g][h] = f1_pool.tile(
                    [128, F1H], BF16, tag="f1g", name=f"f1_{g}_{h}"
                )
            t = f1_tiles[g][h]
            if quarter is None:
                eng.dma_start(t[:], f1r_d[g, h])
            elif quarter == 0:
                eng.dma_start(t[:, : F1H // 2], f1r_d[g, h][:, : F1H // 2])
            else:
                eng.dma_start(t[:, F1H // 2 :], f1r_d[g, h][:, F1H // 2 :])

        # deadline-ordered input streams; each ring is FIFO at ~half the
        # fabric, so per-ring cumulative bytes track each block's deadline.
        # Few DMAs total keeps the 8-sem round-robin pool shallow (fewer
        # recycle chains).
        sy, sc = nc.sync, nc.scalar
        chunk_dma(sy, 0, 0)                  # g0 ch0 window
        chunk_dma(sc, 0, 1)                  # g0 ch1 window
        f1_dma(sy, 0, 0, quarter=0)          # cc0-3 weights: MM0 gate
        f1_dma(sc, 0, 0, quarter=1)          # cc4-7
        f1_dma(sy, 0, 1, quarter=0)          # cc8-11
        f1_dma(sc, 0, 1, quarter=1)          # cc12-15
        chunk_dma(sy, 1, 0)
        chunk_dma(sc, 1, 1)
        f1_dma(sy, 1, 0)
        f1_dma(sc, 1, 1)
        chunk_dma(sy, 2, 0)
        chunk_dma(sc, 2, 1)
        f1_dma(sy, 2, 0)
        f1_dma(sc, 2, 1)
        chunk_dma(sy, 3, 0)
        chunk_dma(sc, 3, 1)
        f1_dma(sy, 3, 0)
        f1_dma(sc, 3, 1)

        def window(g, ch, cc, rlo, rhi):
            # rows 16g+rlo..16g+rhi, x cols 8cc-4..8cc+12 (wrapping into
            # garbage).  The first/last y-blocks skip the window rows that
            # only produce out-of-range (host-masked) displacements.
            start = CHB * ch + PAD + (16 * g + rlo) * W + 8 * cc - PAD
            nr = rhi - rlo
            sl = f2_f[:, start : start + nr * W]
            return sl.rearrange("p (r x) -> p r x", r=nr)[:, :, :WIN]

        for g in range(NG):
            # f1 half layout [cc_local, ch, s, xi]: the (s, xi) weight
            # block for one (cc, ch) is contiguous, as LDWEIGHTS requires
            f1_vs = [
                f1_tiles[g][h][:].rearrange(
                    "p (t c s x) -> p t c (s x)", t=HC, c=2, s=G
                )
                for h in range(2)
            ]
            stage_t = stage_pool.tile(
                [128, NMM * NC_], BF16, tag="stg", name=f"stg_{g}"
            )
            # hybrid layout [row(24), t(16), xw(16)]: evac writes 16-elem
            # contiguous runs; each group-j slab is one contiguous 3072-run
            stage_e = stage_t[:].rearrange("p (r t w) -> p r t w", r=ROWS, t=NC_)

            # skip window rows whose displacements are all out of range
            # (host-masked): 4 at the top of g0, 4 at the bottom of g3
            rlo = PAD if g == 0 else 0
            rhi = ROWS - PAD if g == NG - 1 else ROWS
            nrows = rhi - rlo
            ncols = nrows * WIN

            def mm(cc, ch, dst):
                nc.tensor.matmul(
                    dst,
                    f1_vs[cc // HC][:, cc % HC, ch, :],
                    window(g, ch, cc, rlo, rhi),
                    start=(ch == 0),
                    stop=(ch == 1),
                )

            # two x-blocks share one 2-bank psum tile; a single 4-dim copy
            # evacuates both (halves the per-op fixed overhead, giving the
            # evac lane margin over the PE's production rate)
            def pair_tile(name):
                return psum_pool.tile([128, 1024], F32, tag="ps", name=name)

            def pslice(pt, half):
                return pt[:, 512 * half : 512 * half + ncols]

            def evac_pair(cc, pt):
                base = pt[:]
                src = AP(
                    base.tensor,
                    base.offset,
                    [[1024, 128], [WIN, nrows], [512, 2], [1, WIN]],
                )
                dst = stage_e[:, rlo:rhi, cc : cc + 2, :]
                if (cc // 2) % 2 == 0:
                    nc.vector.tensor_copy(dst, src)
                else:
                    nc.scalar.copy(dst, src)

            if g == 0:
                # ch0 of the first 8 x-blocks first: they gate only on the
                # f1g0a quarters + the ch0 chunk halves, bridging the PE
                # until the ch1 chunk lands (~2.6 us later)
                pts = [pair_tile(f"pp0_{pcc}") for pcc in range(0, 8, 2)]
                for cc in range(8):
                    mm(cc, 0, pslice(pts[cc // 2], cc % 2))
                for cc in range(8):
                    mm(cc, 1, pslice(pts[cc // 2], cc % 2))
                    if cc % 2 == 1:
                        evac_pair(cc - 1, pts[cc // 2])
                rest = range(8, NC_, 2)
            else:
                rest = range(0, NC_, 2)
            for pcc in rest:
                pt = pair_tile(f"pp_{g}_{pcc}")
                mm(pcc, 0, pslice(pt, 0))
                mm(pcc, 1, pslice(pt, 0))
                mm(pcc + 1, 0, pslice(pt, 1))
                mm(pcc + 1, 1, pslice(pt, 1))
                evac_pair(pcc, pt)

            # contiguous per-32-partition-group slabs (group j needs rows
            # [4j, 4j+12), one 3072-elem run per partition).  All on the SP
            # ring except the last y-block, which splits across SP/ACT; the
            # scalar pair is emitted first so any semaphore the allocator
            # recycles between adjacent slabs lands within one engine
            # (where triggers serialize anyway) instead of across engines.
            def slab(eng, j):
                eng.dma_start(
                    s1_d[32 * j : 32 * j + 32, g, :],
                    stage_t[
                        32 * j : 32 * j + 32,
                        64 * j * NC_ : (64 * j + SLAB) * NC_,
                    ],
                )

            if g == NG - 1:
                # split the final 0.79 MB across both rings
                slab(nc.scalar, 1)
                slab(nc.sync, 0)
                slab(nc.scalar, 3)
                slab(nc.sync, 2)
            else:
                for j in range(4):
                    slab(nc.sync, j)

    nc.compile()
    return nc


def prep_inputs(fmap1: np.ndarray, fmap2: np.ndarray):
    f1 = np.asarray(fmap1, dtype=np.float32).reshape(B, 2, 128, NG, G, NC_, A)
    # f1r[b, g, half, cpart, cc_local, ch, s, xi] (cc-major within a half)
    f1r = (
        np.ascontiguousarray(f1.transpose(0, 3, 2, 1, 5, 4, 6))  # b g cp ch cc s xi
        .astype(BF16_NP)
        .reshape(B, NG, 128, 2, 2, HC, G, A)  # b g cp ch half ccl s xi
        .transpose(0, 1, 4, 2, 5, 3, 6, 7)    # b g half cp ccl ch s xi
        .reshape(B, NG, 2, 128, F1H)
    )
    f1r = np.ascontiguousarray(f1r)
    f2 = np.asarray(fmap2, dtype=np.float32).reshape(B, 2, 128, H, W)
    # f2r[b, cpart, ch, y, x] - no padding
    f2r = np.ascontiguousarray(f2.transpose(0, 2, 1, 3, 4)).astype(BF16_NP)
    return f1r, f2r


def _host_gather_idx():
    y = np.arange(H)
    x = np.arange(W)
    g = y // G
    s = y % G
    cc = x // A
    xi = x % A
    p = (8 * s)[:, None] + xi[None, :]          # [H, W]
    dyg = np.arange(NB) // 9
    dxg = np.arange(NB) % 9
    # slab-local offset: (s%4+dy)*(NC_*WIN) + cc*WIN + xi + dx
    e_rel = (
        ((s % 4)[None, :, None] + dyg[:, None, None]) * (NC_ * WIN)
        + cc[None, None, :] * WIN
        + xi[None, None, :]
        + dxg[:, None, None]
    )                                            # [81, H, W]
    flat = (p[None] * NG + g[None, :, None]) * (SLAB * NC_) + e_rel
    # out-of-range displacements correlate against the reference's zero
    # padding -> exactly zero; the kernel leaves garbage there, mask it.
    yy = y[None, :, None] + dyg[:, None, None] - PAD
    xx = x[None, None, :] + dxg[:, None, None] - PAD
    valid = (yy >= 0) & (yy < H) & (xx >= 0) & (xx < W)
    return flat.reshape(-1), valid.reshape(-1)


_FLAT_IDX, _VALID = _host_gather_idx()


def finish_host(s1_all: np.ndarray) -> np.ndarray:
    s1 = np.asarray(s1_all, dtype=np.float32).reshape(B, -1)
    out = np.where(_VALID[None], s1[:, _FLAT_IDX], np.float32(0.0))
    return out.reshape(B, NB, H, W)


_CACHE = {}


def _get_program():
    if "p" not in _CACHE:
        _CACHE["p"] = build_program()
    return _CACHE["p"]


def run_on_cores(fmap1, fmap2, trace=False):
    nc = _get_program()
    f1r, f2r = prep_inputs(fmap1, fmap2)
    in_maps = [{"f1r": f1r[b], "f2": f2r[b]} for b in range(B)]
    res = run_bass_kernel_spmd(nc, in_maps, core_ids=list(range(B)), trace=trace)
    s1_all = np.stack([res.results[b]["s1"] for b in range(B)], axis=0)
    out = finish_host(s1_all)
    return out, res


def kernel(fmap1: np.ndarray, fmap2: np.ndarray) -> np.ndarray:
    fmap1 = np.asarray(fmap1, dtype=np.float32)
    fmap2 = np.asarray(fmap2, dtype=np.float32)
    out, _ = run_on_cores(fmap1, fmap2, trace=False)
    return out
